# revision 30
# baseline (speedup 1.0000x reference)
"""Causal attention (B=2, T=2048, E=1024, H=16, D=64) on 8 TRN2 NeuronCores.

Sharding: core c handles batch b = c//4 and local head group hg = c%4
(4 heads, 256 head-dims).  Data parallel over batch, tensor parallel over
heads; the output projection is row-parallel, so each core returns a
partial [T, E] output and the host sums the 4 partials per batch (bias
is pre-divided by 4 and added on-device).

Device plan (per core, all-bf16 matmuls with fp32 PSUM accumulation):
  xt  = x[b].T                   [E, T]  (host-transposed; e on partitions)
  wqt/wkt/wvt = W[h].T           pre-tiled [P, 8, 256] for SBUF layout
  wpt = Wp[:, h].T               pre-tiled [P, 2, 1024]
  phase 2: q_t/k_t [hd, t] computed directly transposed (lhsT=W.T,
    rhs=xt, contraction over e), ec-outer with 8 open PSUM groups so
    matmuls chase the xt DMA row arrivals; v natural [t, hd] with 64
    ones-columns prepended (softmax denominator comes out of the PV
    matmul replicated on partitions 0:63).
  phase 3: block-causal scores st[j, i] = q_j . k_i with 2-head
    row-packing (two K=64 matmuls in distinct PE row groups), exp on
    ScalarE (scale=1/8; no max subtraction -- scores are ~N(0,1) so
    exp cannot overflow), causal mask multiply only on block-diagonal
    tiles, PV accumulation over j in PSUM, then approx-reciprocal +
    multiply for the normalization.
  phase 4: output projection + bp/4, partial outputs in bf16.
Emission order interleaves head-pair-1 QKV and the projection under
head-pair-0/1 attention so the PE fills ScalarE-bound stretches.
DMA: few large (~0.5-1 MiB) transfers split across both HWDGE rings
(sync + scalar) so SDMA runs near HBM rate from t=0.
"""

import ml_dtypes
import numpy as np

import concourse.bass as bass
import concourse.tile as tile
from concourse import bacc, mybir
from concourse.bass_utils import run_bass_kernel_spmd

B, T, E = 2, 2048, 1024
H, D = 16, 64
NCORES = 8
GROUPS = 4              # cores per batch (tensor parallel over heads)
HL = H // GROUPS        # 4 local heads per core
HDL = HL * D            # 256 local head dims
P = 128
TQ = 512                # i-block (free dim of score tiles)
JB = 128                # j-block (partition dim of score tiles)
N_TB = T // TQ          # 4
N_EC = E // P           # 8
N_TC = T // P           # 16

F32 = mybir.dt.float32
BF16 = mybir.dt.bfloat16
AF = mybir.ActivationFunctionType


def _build_nc():
    nc = bacc.Bacc("TRN2", target_bir_lowering=False, debug=False)
    xt = nc.dram_tensor("xt", [E, T], BF16, kind="ExternalInput").ap()
    wqt = nc.dram_tensor("wqt", [P, N_EC, HDL], BF16, kind="ExternalInput").ap()
    wkt = nc.dram_tensor("wkt", [P, N_EC, HDL], BF16, kind="ExternalInput").ap()
    wvt = nc.dram_tensor("wvt", [P, N_EC, HDL], BF16, kind="ExternalInput").ap()
    wpt = nc.dram_tensor("wpt", [P, 2, E], BF16, kind="ExternalInput").ap()
    bqv = nc.dram_tensor("bqv", [HDL], F32, kind="ExternalInput").ap()
    bkv = nc.dram_tensor("bkv", [HDL], F32, kind="ExternalInput").ap()
    bvv = nc.dram_tensor("bvv", [HDL], F32, kind="ExternalInput").ap()
    bp4 = nc.dram_tensor("bp4", [E], F32, kind="ExternalInput").ap()
    maskd = nc.dram_tensor("mask", [GROUPS, JB, TQ], BF16,
                           kind="ExternalInput").ap()
    onesv = nc.dram_tensor("onesv", [HDL], BF16, kind="ExternalInput").ap()
    out = nc.dram_tensor("out", [T, E], BF16, kind="ExternalOutput").ap()

    with tile.TileContext(nc) as tc:
        with (
            tc.tile_pool(name="big", bufs=1) as big,
            tc.tile_pool(name="work", bufs=5) as work,
            tc.tile_pool(name="outp", bufs=3) as outp,
        ):
            # ---------------- input loads: few big DMAs, 2 HWDGE rings ------
            wq_all = big.tile([P, N_EC, HDL], BF16, tag="wq", name="wq")
            nc.scalar.dma_start(wq_all, wqt)
            wk_all = big.tile([P, N_EC, HDL], BF16, tag="wk", name="wk")
            nc.scalar.dma_start(wk_all, wkt)
            wv_all = big.tile([P, N_EC, HDL], BF16, tag="wv", name="wv")
            nc.scalar.dma_start(wv_all, wvt)
            xt_sb = [big.tile([P, T], BF16, tag=f"xt{ec}", name=f"xt{ec}")
                     for ec in range(N_EC)]
            for ec in range(N_EC):
                nc.sync.dma_start(xt_sb[ec], xt[ec * P:(ec + 1) * P, :])
            # gpsimd (SWDGE): small / late-needed tensors
            bv_sb = big.tile([P, HDL], F32, tag="bv", name="bv")
            nc.gpsimd.dma_start(
                bv_sb, bass.AP(tensor=bvv.tensor, offset=bvv.offset,
                               ap=[[0, P]] + list(bvv.ap)))
            bq_sb = big.tile([P, 2], F32, tag="bq", name="bq")
            nc.gpsimd.dma_start(bq_sb, bqv.rearrange("(c p) -> p c", p=P))
            bk_sb = big.tile([P, 2], F32, tag="bk", name="bk")
            nc.gpsimd.dma_start(bk_sb, bkv.rearrange("(c p) -> p c", p=P))
            ones_sb = big.tile([P, HL, D], BF16, tag="ones", name="ones")
            ones_r = onesv.rearrange("(h d) -> h d", h=HL)
            nc.gpsimd.dma_start(
                ones_sb, bass.AP(tensor=onesv.tensor, offset=onesv.offset,
                                 ap=[[0, P]] + list(ones_r.ap)))
            mask_sb = big.tile([P, GROUPS, TQ], BF16, tag="mask", name="mask")
            nc.gpsimd.dma_start(mask_sb, maskd.rearrange("d p f -> p d f"))
            wp_all = big.tile([P, 2, E], BF16, tag="wp", name="wp")
            nc.gpsimd.dma_start(wp_all, wpt)
            bp_sb = big.tile([P, E], F32, tag="bp", name="bp")
            nc.gpsimd.dma_start(
                bp_sb, bass.AP(tensor=bp4.tensor, offset=bp4.offset,
                               ap=[[0, P]] + list(bp4.ap)))

            q_sb = [big.tile([P, T], BF16, tag=f"q{hc}", name=f"q{hc}")
                    for hc in range(2)]
            k_sb = [big.tile([P, T], BF16, tag=f"k{hc}", name=f"k{hc}")
                    for hc in range(2)]
            at_sb = [big.tile([P, T], BF16, tag=f"at{hc}", name=f"at{hc}")
                     for hc in range(2)]
            v_sb = [big.tile([P, HL, 2 * D], BF16, tag=f"v{t}", name=f"v{t}")
                    for t in range(N_TC)]

            # -------- phase 2 (bf16): q/k for head-pair hc, v; ec-outer -----
            def qk_phase(ph2ps, hc):
                # 8 concurrently-open PSUM groups (q + k for one head pair):
                # every xt row arrival unlocks one matmul per group.
                pss = [ph2ps.tile([P, TQ], F32, tag="mm", name="mm")
                       for _ in range(8)]
                for ec in range(N_EC):
                    for tb in range(N_TB):
                        for wi, w_all in enumerate((wq_all, wk_all)):
                            nc.tensor.matmul(
                                pss[tb * 2 + wi],
                                lhsT=w_all[:, ec, hc * P:(hc + 1) * P],
                                rhs=xt_sb[ec][:, tb * TQ:(tb + 1) * TQ],
                                start=(ec == 0), stop=(ec == N_EC - 1))
                for tb in range(N_TB):
                    for wi, (bias_t, dst) in enumerate(((bq_sb, q_sb),
                                                        (bk_sb, k_sb))):
                        nc.vector.tensor_scalar_add(
                            dst[hc][:, tb * TQ:(tb + 1) * TQ],
                            pss[tb * 2 + wi], bias_t[:, hc:hc + 1])

            def v_phase(ph2ps, waves=(0, 1)):
                for wave in waves:
                    pss = [ph2ps.tile([P, HDL], F32, tag="mm", name="mm")
                           for _ in range(8)]
                    for ec in range(N_EC):
                        for ti in range(8):
                            t_ = wave * 8 + ti
                            nc.tensor.matmul(
                                pss[ti],
                                lhsT=xt_sb[ec][:, t_ * P:(t_ + 1) * P],
                                rhs=wv_all[:, ec, :],
                                start=(ec == 0), stop=(ec == N_EC - 1))
                    for ti in range(8):
                        t_ = wave * 8 + ti
                        nc.vector.tensor_copy(v_sb[t_][:, :, 0:D], ones_sb)
                        nc.vector.tensor_add(
                            v_sb[t_][:, :, D:2 * D],
                            pss[ti].rearrange("p (h d) -> p h d", h=HL),
                            bv_sb.rearrange("p (h d) -> p h d", h=HL))

            # -------- phase 3 (bf16): block-causal attention per head pair --
            def attention(stps, accps, hp, ibs=None):
                for ib in (range(N_TB) if ibs is None else ibs):
                    njb = 4 * ib + 4
                    accs = [accps.tile([2 * D, TQ], F32, tag=f"acc{h}",
                                       name=f"acc{h}") for h in range(2)]
                    for jb in range(njb):       # j blocks of 128
                        idx = jb - 4 * ib       # >= 0 on the block diagonal
                        dd = idx * JB if idx >= 0 else 0
                        st = stps.tile([P, 2, TQ], F32, tag="st", name="st")
                        pt = work.tile([P, 2, TQ], BF16, tag="pt", name="pt")
                        for h in range(2):
                            pr = slice(h * D, (h + 1) * D)
                            nc.tensor.matmul(
                                st[:, h, dd:],
                                lhsT=q_sb[hp][pr, jb * JB:(jb + 1) * JB],
                                rhs=k_sb[hp][pr, ib * TQ + dd:(ib + 1) * TQ],
                                start=True, stop=True)
                        nc.scalar.activation(pt[:, :, dd:], st[:, :, dd:],
                                             AF.Exp, scale=0.125)
                        if idx >= 0:
                            for h in range(2):
                                nc.vector.tensor_mul(
                                    pt[:, h, dd:], pt[:, h, dd:],
                                    mask_sb[:, idx, dd:])
                        for h in range(2):
                            nc.tensor.matmul(
                                accs[h][:, dd:],
                                lhsT=v_sb[jb][:, 2 * hp + h, :],
                                rhs=pt[:, h, dd:],
                                start=(jb == 0), stop=(jb == njb - 1))
                    for h in range(2):
                        rec64 = work.tile([D, TQ], F32, tag="rec64",
                                          name="rec64")
                        nc.vector.reciprocal_approx_fast(rec64,
                                                         accs[h][0:D, :])
                        nc.vector.tensor_mul(
                            at_sb[hp][h * D:(h + 1) * D,
                                      ib * TQ:(ib + 1) * TQ],
                            accs[h][D:2 * D, :], rec64)

            def proj_phase(mmps, ts=None):
                for t_ in (range(N_TC) if ts is None else ts):
                    ot = outp.tile([P, E], BF16, tag="ot", name="ot")
                    for eb in range(2):
                        ps = mmps.tile([P, TQ], F32, tag="mm", name="mm")
                        for hc in range(2):
                            nc.tensor.matmul(
                                ps,
                                lhsT=at_sb[hc][:, t_ * P:(t_ + 1) * P],
                                rhs=wp_all[:, hc, eb * TQ:(eb + 1) * TQ],
                                start=(hc == 0), stop=(hc == 1))
                        nc.vector.tensor_add(
                            ot[:, eb * TQ:(eb + 1) * TQ], ps,
                            bp_sb[:, eb * TQ:(eb + 1) * TQ])
                    nc.sync.dma_start(out[t_ * P:(t_ + 1) * P, :], ot)

            def v_phase2(mmps):
                # wave 1 (t 8..15) as fill work on the 2-slot mm pool
                for ti in range(8):
                    t_ = 8 + ti
                    ps = mmps.tile([P, HDL], F32, tag="mm", name="mm")
                    for ec in range(N_EC):
                        nc.tensor.matmul(
                            ps,
                            lhsT=xt_sb[ec][:, t_ * P:(t_ + 1) * P],
                            rhs=wv_all[:, ec, :],
                            start=(ec == 0), stop=(ec == N_EC - 1))
                    nc.vector.tensor_copy(v_sb[t_][:, :, 0:D], ones_sb)
                    nc.vector.tensor_add(
                        v_sb[t_][:, :, D:2 * D],
                        ps.rearrange("p (h d) -> p h d", h=HL),
                        bv_sb.rearrange("p (h d) -> p h d", h=HL))

            # ---- orchestration: overlap hp0 attention with hc1 q/k + vB ----
            import contextlib
            with tc.tile_pool(name="ph2ps", bufs=8, space="PSUM") as ph2ps:
                qk_phase(ph2ps, 0)
                v_phase(ph2ps, waves=(0,))
            _ph34 = contextlib.ExitStack()
            stps = _ph34.enter_context(
                tc.tile_pool(name="stps", bufs=2, space="PSUM"))
            accps = _ph34.enter_context(
                tc.tile_pool(name="accps", bufs=1, space="PSUM"))
            mmps = _ph34.enter_context(
                tc.tile_pool(name="mmps", bufs=2, space="PSUM"))
            def qk_phase2(hc, tbs):
                # hc=1 q/k using the 2-slot mm pool (runs under hp0 attention)
                for tb in tbs:
                    for wi, (w_all, bias_t, dst) in enumerate(
                            ((wq_all, bq_sb, q_sb), (wk_all, bk_sb, k_sb))):
                        ps = mmps.tile([P, TQ], F32, tag="mm", name="mm")
                        for ec in range(N_EC):
                            nc.tensor.matmul(
                                ps,
                                lhsT=w_all[:, ec, hc * P:(hc + 1) * P],
                                rhs=xt_sb[ec][:, tb * TQ:(tb + 1) * TQ],
                                start=(ec == 0), stop=(ec == N_EC - 1))
                        nc.vector.tensor_scalar_add(
                            dst[hc][:, tb * TQ:(tb + 1) * TQ], ps,
                            bias_t[:, hc:hc + 1])

            attention(stps, accps, 0, ibs=[0, 1])
            v_phase2(mmps)
            qk_phase2(1, tbs=[0, 1])
            for ib in (2, 3):
                attention(stps, accps, 0, ibs=[ib])
                qk_phase2(1, tbs=[ib])
            for ib in range(N_TB):
                attention(stps, accps, 1, ibs=[ib])
                proj_phase(mmps, ts=range(ib * 4, ib * 4 + 4))
            _ph34.close()

    nc.compile()
    return nc


def _make_mask():
    jj = np.arange(JB)[:, None]
    ii = np.arange(TQ)[None, :]
    m = np.zeros((GROUPS, JB, TQ), dtype=np.float32)
    for d in range(GROUPS):
        m[d] = (jj + d * JB <= ii).astype(np.float32)
    return m.astype(ml_dtypes.bfloat16)


_NC = None


def _get_nc():
    global _NC
    if _NC is None:
        _NC = _build_nc()
    return _NC


def _warr(w):
    """W slice [HDL, E] -> SBUF layout [P, N_EC, HDL]: element (p, c, f) =
    W.T[c*P + p, f]."""
    return np.ascontiguousarray(
        w.T.reshape(N_EC, P, HDL).transpose(1, 0, 2)).astype(ml_dtypes.bfloat16)


def kernel(x, Wq, bq, Wk, bk, Wv, bv, Wp, bp, **_run_kwargs):
    x = np.asarray(x, dtype=np.float32)
    Wq = np.asarray(Wq, dtype=np.float32)
    Wk = np.asarray(Wk, dtype=np.float32)
    Wv = np.asarray(Wv, dtype=np.float32)
    Wp = np.asarray(Wp, dtype=np.float32)
    bq = np.asarray(bq, dtype=np.float32)
    bk = np.asarray(bk, dtype=np.float32)
    bv = np.asarray(bv, dtype=np.float32)
    bp = np.asarray(bp, dtype=np.float32)

    mask = _make_mask()
    bp4 = (bp / GROUPS).astype(np.float32)

    in_maps = []
    for c in range(NCORES):
        b, hg = divmod(c, GROUPS)
        hsl = slice(HDL * hg, HDL * (hg + 1))
        in_maps.append({
            "xt": np.ascontiguousarray(x[b].T).astype(ml_dtypes.bfloat16),
            "wqt": _warr(Wq[hsl]),
            "wkt": _warr(Wk[hsl]),
            "wvt": _warr(Wv[hsl]),
            "wpt": np.ascontiguousarray(
                Wp[:, hsl].T.reshape(2, P, E).transpose(1, 0, 2)
            ).astype(ml_dtypes.bfloat16),
            "bqv": np.ascontiguousarray(bq[hsl]),
            "bkv": np.ascontiguousarray(bk[hsl]),
            "bvv": np.ascontiguousarray(bv[hsl]),
            "bp4": bp4,
            "mask": mask,
            "onesv": np.ones(HDL, dtype=ml_dtypes.bfloat16),
        })

    nc = _get_nc()
    try:
        res = run_bass_kernel_spmd(nc, in_maps, core_ids=list(range(NCORES)),
                                   **_run_kwargs)
    except Exception:
        # transient device hiccups (e.g. NRT_EXEC_UNIT_UNRECOVERABLE) have
        # been observed to clear on retry
        import time
        time.sleep(2.0)
        res = run_bass_kernel_spmd(nc, in_maps, core_ids=list(range(NCORES)),
                                   **_run_kwargs)
    outs = [r["out"].astype(np.float32) for r in res.results]
    y = np.stack([
        outs[0] + outs[1] + outs[2] + outs[3],
        outs[4] + outs[5] + outs[6] + outs[7],
    ]).astype(np.float32)
    if _run_kwargs:
        return y, res
    return y
